# revision 1
# baseline (speedup 1.0000x reference)
"""Local (windowed) attention kernel for Trainium2, sequence-parallel over 8 NeuronCores.

Reference computation (fp32):
    qkv = x @ w_qkv ; q,k,v split, reshaped to (head, window, 128, 64)
    k,v get a 1-window zero-padded lookback -> (head, window, 256, 64)
    sim = q @ k.T * d^-0.5, causal-banded mask, softmax, out = attn @ v
    y = out @ w_out + b_out

Sharding: 128 windows of 128 tokens -> 16 windows per core, plus a 128-row
halo of x from the previous core (zeros for core 0, which exactly reproduces
the reference's zero-pad lookback including its effect on the softmax
denominator). No inter-core communication.

Device dataflow (per core, all bf16 matmuls accumulating in fp32):
  xT (host-pretransposed, [1024, 2176]) and w_qkv stream in; qkT = w_q/k.T @ xT
  keeps features on partitions, v = xT.T @ w_v keeps tokens on partitions with
  a ones-column appended per head (so attn@v also yields the softmax
  denominator for free). Scores are computed transposed (pT[j, i]) so that
  attn@v needs no on-device transposes and its output lands directly as the
  stationary operand of the output projection. Softmax skips max-subtraction
  (logits are ~N(0, 0.4); exp is safe in fp32).
"""

import sys

sys.path.insert(0, "/opt/trn_rl_repo")

import numpy as np
import ml_dtypes

import concourse.bass as bass
import concourse.mybir as mybir
import concourse.tile as tile
from concourse import bacc
from concourse.bass_utils import run_bass_kernel_spmd

BF16 = mybir.dt.bfloat16
F32 = mybir.dt.float32

N = 16384
DIM = 1024
HEADS = 8
DHEAD = 64
WSZ = 128
NCORES = 8
R = N // NCORES            # 2048 own rows per core
T = R + WSZ                # 2176 rows incl. halo
NW = R // WSZ              # 16 own windows
DK = DIM // 128            # 8 contraction chunks
P = 128
SCALE = DHEAD ** -0.5

_CACHE = {}


def _build():
    nc = bacc.Bacc()
    xT_d = nc.declare_dram_parameter("xT", [DIM, T], BF16, isOutput=False)
    wqkv_d = nc.declare_dram_parameter("wqkv", [DIM, 3 * HEADS * DHEAD], BF16, isOutput=False)
    wout_d = nc.declare_dram_parameter("wout", [HEADS * DHEAD, DIM], BF16, isOutput=False)
    maskT_d = nc.declare_dram_parameter("maskT", [P, P], BF16, isOutput=False)
    ones1_d = nc.declare_dram_parameter("ones1", [1, DHEAD], BF16, isOutput=False)
    out_d = nc.declare_dram_parameter("out", [R, DIM], F32, isOutput=True)

    # token blocks for the qkT projection (moving dim <= 512)
    tok_blocks = [(b, min(512, T - b)) for b in range(0, T, 512)]

    with tile.TileContext(nc) as tc:
        with (
            tc.tile_pool(name="pers", bufs=1) as pers,
            tc.tile_pool(name="work", bufs=3) as work,
            tc.tile_pool(name="outp", bufs=2) as outp,
            tc.tile_pool(name="ps512", bufs=2, space="PSUM") as ps512,
            tc.tile_pool(name="pspt", bufs=2, space="PSUM") as pspt,
            tc.tile_pool(name="pso", bufs=2, space="PSUM") as pso,
            tc.tile_pool(name="psb", bufs=2, space="PSUM") as psb,
        ):
            # ---- phase A: load inputs -------------------------------------
            xT_sb = [pers.tile([P, T], BF16, tag=f"xT{k}", name=f"xT{k}") for k in range(DK)]
            w_sb = [pers.tile([P, 3 * HEADS * DHEAD], BF16, tag=f"w{k}", name=f"w{k}") for k in range(DK)]
            wo_sb = [pers.tile([P, DIM], BF16, tag=f"wo{m}", name=f"wo{m}") for m in range(4)]
            maskT_sb = pers.tile([P, P], BF16, tag="maskT")
            ones1_sb = pers.tile([1, DHEAD], BF16, tag="ones1")
            for k in range(DK):
                nc.sync.dma_start(xT_sb[k][:], xT_d[k * P:(k + 1) * P, :])
                nc.sync.dma_start(w_sb[k][:], wqkv_d[k * P:(k + 1) * P, :])
            for m in range(4):
                nc.sync.dma_start(wo_sb[m][:], wout_d[m * P:(m + 1) * P, :])
            nc.sync.dma_start(maskT_sb[:], maskT_d[:])
            nc.sync.dma_start(ones1_sb[:], ones1_d[:])

            # ---- phase B: qkT[m] = w_qk[:, m-chunk].T @ xT  ([128, T]) ----
            qk_sb = [pers.tile([P, T], BF16, tag=f"qk{m}", name=f"qk{m}") for m in range(8)]
            for m in range(8):
                for (b0, bw) in tok_blocks:
                    pq = ps512.tile([P, 512], F32, tag="mm512", name="mm512")
                    for k in range(DK):
                        nc.tensor.matmul(
                            pq[:, :bw],
                            lhsT=w_sb[k][:, m * P:(m + 1) * P],
                            rhs=xT_sb[k][:, b0:b0 + bw],
                            start=(k == 0), stop=(k == DK - 1),
                        )
                    nc.vector.tensor_copy(qk_sb[m][:, b0:b0 + bw], pq[:, :bw])

            # ---- phase C: v[t] = xT[:, t-tile].T @ w_v  (+ ones column) ---
            # v_sb[t] is [128 tok, 8 heads, 65]; [:, h, 0:64] = v, [:, h, 64] = 1
            v_sb = [pers.tile([P, HEADS, DHEAD + 1], BF16, tag=f"v{t}", name=f"v{t}") for t in range(NW + 1)]
            for t in range(NW + 1):
                nc.vector.memset(v_sb[t][:, :, DHEAD:DHEAD + 1], 1.0)
                pv = ps512.tile([P, 512], F32, tag="mm512", name="mm512")
                for k in range(DK):
                    nc.tensor.matmul(
                        pv[:],
                        lhsT=xT_sb[k][:, t * P:(t + 1) * P],
                        rhs=w_sb[k][:, 1024:1536],
                        start=(k == 0), stop=(k == DK - 1),
                    )
                nc.vector.tensor_copy(
                    v_sb[t][:, :, 0:DHEAD],
                    pv.rearrange("p (h d) -> p h d", h=HEADS),
                )

            # ---- phase D: attention per (window, head) --------------------
            # attn_sb[m] rows 0:64 = head 2m, 64:128 = head 2m+1 (out.T layout)
            attn_sb = [pers.tile([P, R], BF16, tag=f"at{m}", name=f"at{m}") for m in range(4)]
            for w in range(NW):
                for h in range(HEADS):
                    mq, off = h // 2, (h % 2) * 64
                    mk = 4 + h // 2
                    i0 = (w + 1) * P
                    ppt = pspt.tile([P, 2, P], F32, tag="pT", name="pT")
                    # scores transposed: pT[j, i] for j in prev/cur window
                    for jc in range(2):
                        j0 = (w + jc) * P
                        nc.tensor.matmul(
                            ppt[:, jc, :],
                            lhsT=qk_sb[mk][off:off + 64, j0:j0 + P],
                            rhs=qk_sb[mq][off:off + 64, i0:i0 + P],
                            start=True, stop=True,
                        )
                    pt_sb = work.tile([P, 2, P], BF16, tag="pt_sb", name="pt_sb")
                    nc.scalar.activation(pt_sb[:], ppt[:],
                                         mybir.ActivationFunctionType.Exp, scale=SCALE)
                    # causal mask inside the current window (prev window is
                    # fully visible: j <= i + 128 always holds there)
                    nc.vector.tensor_mul(pt_sb[:, 1, :], pt_sb[:, 1, :], maskT_sb[:])
                    # attn @ v (+ denominator in row 64, from the ones column)
                    po = pso.tile([DHEAD + 1, P], F32, tag="o", name="po")
                    for jc in range(2):
                        nc.tensor.matmul(
                            po[:],
                            lhsT=v_sb[w + jc][:, h, :],
                            rhs=pt_sb[:, jc, :],
                            start=(jc == 0), stop=(jc == 1),
                        )
                    r_sb = work.tile([1, P], BF16, tag="r_sb", name="r_sb")
                    with nc.allow_low_precision(reason="softmax denom recip in bf16"):
                        nc.vector.reciprocal(r_sb[:], po[DHEAD:DHEAD + 1, :])
                    # broadcast recip across 64 partitions via K=1 outer product
                    pb = psb.tile([DHEAD, P], F32, tag="b", name="pb")
                    nc.tensor.matmul(pb[:], lhsT=ones1_sb[:], rhs=r_sb[:],
                                     start=True, stop=True)
                    b_sb = work.tile([DHEAD, P], F32, tag="b_sb", name="b_sb")
                    nc.scalar.copy(b_sb[:], pb[:])
                    nc.vector.tensor_mul(
                        attn_sb[mq][off:off + 64, w * P:(w + 1) * P],
                        po[0:DHEAD, :], b_sb[:],
                    )

            # ---- phase E: out = attn.T @ w_out ----------------------------
            for t in range(NW):
                o_sb = outp.tile([P, DIM], F32, tag="o_sb", name="o_sb")
                for nf in range(2):
                    pf = ps512.tile([P, 512], F32, tag="mm512", name="mm512")
                    for m in range(4):
                        nc.tensor.matmul(
                            pf[:],
                            lhsT=attn_sb[m][:, t * P:(t + 1) * P],
                            rhs=wo_sb[m][:, nf * 512:(nf + 1) * 512],
                            start=(m == 0), stop=(m == 3),
                        )
                    nc.vector.tensor_copy(o_sb[:, nf * 512:(nf + 1) * 512], pf[:])
                nc.sync.dma_start(out_d[t * P:(t + 1) * P, :], o_sb[:])

    nc.compile()
    return nc


def _get_nc():
    if "nc" not in _CACHE:
        _CACHE["nc"] = _build()
    return _CACHE["nc"]


def kernel(x, w_qkv, w_out, b_out):
    x = np.asarray(x, dtype=np.float32)
    w_qkv_b = np.asarray(w_qkv, dtype=np.float32).astype(ml_dtypes.bfloat16)
    w_out_b = np.asarray(w_out, dtype=np.float32).astype(ml_dtypes.bfloat16)
    b_out = np.asarray(b_out, dtype=np.float32)

    # maskT[j, i] = 1 where j <= i  (transposed causal mask for current window)
    maskT = np.triu(np.ones((P, P), dtype=np.float32)).astype(ml_dtypes.bfloat16)
    ones1 = np.ones((1, DHEAD), dtype=ml_dtypes.bfloat16)

    x_pad = np.concatenate([np.zeros((WSZ, DIM), np.float32), x], axis=0)
    in_maps = []
    for c in range(NCORES):
        x_sh = x_pad[c * R:c * R + T]                       # (2176, 1024)
        xT = np.ascontiguousarray(x_sh.T).astype(ml_dtypes.bfloat16)
        in_maps.append({
            "xT": xT,
            "wqkv": w_qkv_b,
            "wout": w_out_b,
            "maskT": maskT,
            "ones1": ones1,
        })

    nc = _get_nc()
    res = run_bass_kernel_spmd(nc, in_maps, core_ids=list(range(NCORES)))
    out = np.concatenate([res.results[c]["out"] for c in range(NCORES)], axis=0)
    return out + b_out[None, :]



# revision 10
# speedup vs baseline: 1.1244x; 1.1244x over previous
"""Local (windowed) attention kernel for Trainium2, sequence-parallel over 8 NeuronCores.

Reference computation (fp32):
    qkv = x @ w_qkv ; q,k,v split, reshaped to (head, window, 128, 64)
    k,v get a 1-window zero-padded lookback -> (head, window, 256, 64)
    sim = q @ k.T * d^-0.5, causal-banded mask, softmax, out = attn @ v
    y = out @ w_out + b_out

Sharding: 128 windows of 128 tokens -> 16 windows per core, plus a 128-row
halo of x from the previous core (zeros for core 0, which exactly reproduces
the reference's zero-pad lookback including its effect on the softmax
denominator). No inter-core communication.

Device dataflow (per core, bf16 matmuls accumulating in fp32):
  B: qkT[m] = w_qk[:,m].T @ xT keeps q/k features on partitions.
  C: v[t] = xT[:,t].T @ w_v keeps tokens on partitions; columns 64:128 of each
     per-head v tile are ones, so attn@v replicates the softmax denominator
     onto PSUM partitions 64:128 (no separate reduction needed).
  D: per key-window kw, one N=256 score matmul per head covers both query
     windows that attend kw; the causal mask is added in PSUM via a
     maskadd.T @ I matmul (exp(-30) == 0), so the Scalar-engine exp needs no
     follow-up masking. Softmax skips max-subtraction (logits are small).
     attn@v accumulates each query block's output over its two key windows;
     normalization is one 1024-element reciprocal plus 8 multiplies per
     window on DVE.
  E: out.T @ w_out interleaved two iterations behind D so the Tensor engine
     never idles (idle gaps halve the PE clock via its p-state ramp).
"""

import sys

sys.path.insert(0, "/opt/trn_rl_repo")

import numpy as np
import ml_dtypes

import concourse.bass as bass
import concourse.mybir as mybir
import concourse.tile as tile
from concourse import bacc
from concourse.bass_utils import run_bass_kernel_spmd

BF16 = mybir.dt.bfloat16
F32 = mybir.dt.float32

N = 16384
DIM = 1024
HEADS = 8
DHEAD = 64
WSZ = 128
NCORES = 8
R = N // NCORES            # 2048 own rows per core
T = R + WSZ                # 2176 rows incl. halo
NW = R // WSZ              # 16 own windows
NKW = NW + 1               # 17 key windows (incl. halo window 0)
DK = DIM // 128            # 8 contraction chunks
P = 128
SCALE = DHEAD ** -0.5
MASKVAL = -240.0           # exp(0.125 * -240) == 0 in fp32

_CACHE = {}


def _build():
    nc = bacc.Bacc()
    xT_d = nc.declare_dram_parameter("xT", [DIM, T], BF16, isOutput=False)
    wqkv_d = nc.declare_dram_parameter("wqkv", [DIM, 3 * HEADS * DHEAD], BF16, isOutput=False)
    wout_d = nc.declare_dram_parameter("wout", [HEADS * DHEAD, DIM], BF16, isOutput=False)
    maskadd_d = nc.declare_dram_parameter("maskadd", [P, P], BF16, isOutput=False)
    ident_d = nc.declare_dram_parameter("ident", [P, 256], BF16, isOutput=False)
    out_d = nc.declare_dram_parameter("out", [R, DIM], F32, isOutput=True)

    tok_blocks = [(b, min(512, T - b)) for b in range(0, T, 512)]

    with tile.TileContext(nc) as tc:
        with (
            tc.tile_pool(name="pers", bufs=1) as pers,
            tc.tile_pool(name="ptp", bufs=3) as ptp,
            tc.tile_pool(name="rsp", bufs=2) as rsp,
            tc.tile_pool(name="osb", bufs=2) as osb,
            tc.tile_pool(name="big", bufs=2, space="PSUM") as big,
            tc.tile_pool(name="scp", bufs=2, space="PSUM") as scp,
            tc.tile_pool(name="pop", bufs=2, space="PSUM") as pop,
        ):
            # ---- inputs ---------------------------------------------------
            xT_sb = [pers.tile([P, T], BF16, tag=f"xT{k}", name=f"xT{k}") for k in range(DK)]
            w_sb = [pers.tile([P, 3 * HEADS * DHEAD], BF16, tag=f"w{k}", name=f"w{k}") for k in range(DK)]
            wo_sb = [pers.tile([P, DIM], BF16, tag=f"wo{m}", name=f"wo{m}") for m in range(4)]
            maskadd_sb = pers.tile([P, P], BF16, tag="maskadd")
            ident_sb = pers.tile([P, 256], BF16, tag="ident")
            for k in range(DK):
                nc.sync.dma_start(w_sb[k][:], wqkv_d[k * P:(k + 1) * P, :])
            for (b0, bw) in tok_blocks:
                for k in range(DK):
                    nc.sync.dma_start(xT_sb[k][:, b0:b0 + bw], xT_d[k * P:(k + 1) * P, b0:b0 + bw])
            for m in range(4):
                nc.sync.dma_start(wo_sb[m][:], wout_d[m * P:(m + 1) * P, :])
            nc.sync.dma_start(maskadd_sb[:], maskadd_d[:])
            nc.sync.dma_start(ident_sb[:], ident_d[:])

            # v tiles: [tok, head, 0:64]=v, [tok, head, 64:128]=1 (denominator)
            v_sb = [pers.tile([P, HEADS, P], BF16, tag=f"v{t}", name=f"v{t}") for t in range(NKW)]
            for t in range(NKW):
                nc.vector.memset(v_sb[t][:, :, DHEAD:P], 1.0)

            # ---- phase B: qkT[m] = w_qk[:, m-chunk].T @ xT  ([128, T]) ----
            qk_sb = [pers.tile([P, T], BF16, tag=f"qk{m}", name=f"qk{m}") for m in range(8)]
            for m in range(8):
                for (b0, bw) in tok_blocks:
                    pq = big.tile([P, 512], F32, tag="big", name="bigB")
                    for k in range(DK):
                        nc.tensor.matmul(
                            pq[:, :bw],
                            lhsT=w_sb[k][:, m * P:(m + 1) * P],
                            rhs=xT_sb[k][:, b0:b0 + bw],
                            start=(k == 0), stop=(k == DK - 1),
                        )
                    nc.vector.tensor_copy(qk_sb[m][:, b0:b0 + bw], pq[:, :bw])

            # ---- phase C: v[t] = xT[:, t-tile].T @ w_v --------------------
            for t in range(NKW):
                pv = big.tile([P, 512], F32, tag="big", name="bigC")
                for k in range(DK):
                    nc.tensor.matmul(
                        pv[:],
                        lhsT=xT_sb[k][:, t * P:(t + 1) * P],
                        rhs=w_sb[k][:, 1024:1536],
                        start=(k == 0), stop=(k == DK - 1),
                    )
                nc.vector.tensor_copy(
                    v_sb[t][:, :, 0:DHEAD],
                    pv.rearrange("p (h d) -> p h d", h=HEADS),
                )

            # ---- fused D (attention) + E (out proj) over key windows -----
            # pt[kw] covers query cols [kw*128, kw*128+256): first half is
            # query block kw attending kw as its own (masked) window, second
            # half is query block kw+1 attending kw as its previous window.
            # po[Qb] accumulates attn@v for query block Qb over its two key
            # windows; row 64:128 of each head slice is the softmax denom.
            attn_sb = [pers.tile([P, R], BF16, tag=f"at{m}", name=f"at{m}") for m in range(4)]
            pt_tiles = {}
            po_tiles = {}

            for i in range(NKW + 2):
                kw = i
                # -- scores + exp for key window kw --
                if kw < NKW:
                    # full 256-wide computation even for kw=0 (the halo query
                    # block's scores are valid but unused) so every psum
                    # group opens at offset 0 — a start=True first-touch at a
                    # mid-bank offset wedges the hardware.
                    q0 = kw * P
                    qn = min((kw + 2) * P, T)
                    qw = qn - q0
                    pt = ptp.tile([P, HEADS, 256], BF16, tag="pt", name=f"pt{kw}")
                    pt_tiles[kw] = pt
                    for s in range(4):
                        ppt = scp.tile([P, 2, 256], F32, tag="sc", name=f"sc{kw}_{s}")
                        for hh in range(2):
                            # open the psum group with the causal mask:
                            # psum[j, i] = maskadd[i, j] (-240 above the
                            # diagonal of the own-window block, 0 in the
                            # prev-window half via the zero columns of identz)
                            nc.tensor.matmul(
                                ppt[:, hh, 0:qw],
                                lhsT=maskadd_sb[:],
                                rhs=ident_sb[:, 0:qw],
                                start=True, stop=False,
                            )
                            nc.tensor.matmul(
                                ppt[:, hh, 0:qw],
                                lhsT=qk_sb[4 + s][hh * 64:(hh + 1) * 64, kw * P:(kw + 1) * P],
                                rhs=qk_sb[s][hh * 64:(hh + 1) * 64, q0:qn],
                                start=False, stop=True,
                            )
                        nc.scalar.activation(
                            pt[:, 2 * s:2 * s + 2, 0:qw],
                            ppt[:, :, 0:qw],
                            mybir.ActivationFunctionType.Exp, scale=SCALE,
                        )

                # -- attn@v for query block Qb = i-1 --
                # Both key-window contributions are issued back-to-back per
                # head so each PSUM accumulation group opens and closes within
                # the same bank before the next head's group starts (PSUM
                # start=True zeroing is bank-granular).
                if 2 <= i <= NKW:
                    qb = i - 1
                    po = pop.tile([P, HEADS, P], F32, tag="po", name=f"po{qb}")
                    for h in range(HEADS):
                        # prev-window term from pt[qb-1], own-window from pt[qb]
                        nc.tensor.matmul(
                            po[:, h, :],
                            lhsT=v_sb[qb - 1][:, h, :],
                            rhs=pt_tiles[qb - 1][:, h, P:256],
                            start=True, stop=False,
                        )
                        nc.tensor.matmul(
                            po[:, h, :],
                            lhsT=v_sb[qb][:, h, :],
                            rhs=pt_tiles[qb][:, h, 0:P],
                            start=False, stop=True,
                        )
                    # -- normalize query block qb into attn_sb --
                    rs = rsp.tile([DHEAD, HEADS, P], BF16, tag="rs", name=f"rs{qb}")
                    with nc.allow_low_precision(reason="softmax denom recip in bf16"):
                        nc.vector.reciprocal(rs[:], po[DHEAD:P, :, :])
                    c0 = (qb - 1) * P
                    for h in range(HEADS):
                        m, off = h // 2, (h % 2) * 64
                        nc.vector.tensor_mul(
                            attn_sb[m][off:off + 64, c0:c0 + P],
                            po[0:DHEAD, h, :],
                            rs[:, h, :],
                        )

                # -- out projection for query block i-2 --
                if i >= 3:
                    qb = i - 2
                    c0 = (qb - 1) * P
                    o_sb = osb.tile([P, DIM], F32, tag="o_sb", name=f"o{qb}")
                    for nf in range(2):
                        pf = big.tile([P, 512], F32, tag="big", name=f"bigE{qb}_{nf}")
                        for m in range(4):
                            nc.tensor.matmul(
                                pf[:],
                                lhsT=attn_sb[m][:, c0:c0 + P],
                                rhs=wo_sb[m][:, nf * 512:(nf + 1) * 512],
                                start=(m == 0), stop=(m == 3),
                            )
                        nc.scalar.copy(o_sb[:, nf * 512:(nf + 1) * 512], pf[:])
                    nc.sync.dma_start(out_d[c0:c0 + P, :], o_sb[:])

    nc.compile()
    return nc


def _get_nc():
    if "nc" not in _CACHE:
        _CACHE["nc"] = _build()
    return _CACHE["nc"]


def make_inputs(x, w_qkv, w_out):
    """Host-side prep of the per-core DRAM parameter maps."""
    w_qkv_b = np.asarray(w_qkv, dtype=np.float32).astype(ml_dtypes.bfloat16)
    w_out_b = np.asarray(w_out, dtype=np.float32).astype(ml_dtypes.bfloat16)
    # maskadd[i, j] = -240 where j > i (adds to psum[j, i] via maskadd.T @ I)
    maskadd = (MASKVAL * np.triu(np.ones((P, P), np.float32), 1)).astype(ml_dtypes.bfloat16)
    ident = np.concatenate([np.eye(P, dtype=np.float32),
                            np.zeros((P, P), np.float32)], axis=1).astype(ml_dtypes.bfloat16)
    x_pad = np.concatenate([np.zeros((WSZ, DIM), np.float32), np.asarray(x, np.float32)], axis=0)
    in_maps = []
    for c in range(NCORES):
        x_sh = x_pad[c * R:c * R + T]                       # (2176, 1024)
        xT = np.ascontiguousarray(x_sh.T).astype(ml_dtypes.bfloat16)
        in_maps.append({
            "xT": xT,
            "wqkv": w_qkv_b,
            "wout": w_out_b,
            "maskadd": maskadd,
            "ident": ident,
        })
    return in_maps


def kernel(x, w_qkv, w_out, b_out):
    b_out = np.asarray(b_out, dtype=np.float32)
    in_maps = make_inputs(x, w_qkv, w_out)
    nc = _get_nc()
    res = run_bass_kernel_spmd(nc, in_maps, core_ids=list(range(NCORES)))
    out = np.concatenate([res.results[c]["out"] for c in range(NCORES)], axis=0)
    return out + b_out[None, :]
